# revision 45
# baseline (speedup 1.0000x reference)
"""Trainium2 Bass kernel for nn_AttentionBlock (B=4, C=256, H=W=64, IC=128).

Sharding: 8 cores = 4 batches x 2 row-halves of the N=4096 attention dim.
Each core computes its 2048 rows of the attention output, the final 1x1 conv
(wy), and partial BatchNorm statistics; a tiny AllReduce combines the BN
stats; each core then applies BN + residual and writes its output slice.

Algebraic simplifications vs the reference (all exact):
  - g_b and w_b only add a per-channel constant to wy, which BatchNorm's
    mean subtraction cancels -> dropped.
  - dy_b (phi bias) only adds row-constant terms to the attention logits,
    which softmax cancels -> dropped. Only dx_b (theta bias) is applied
    (folded into the theta PSUM->SBUF copy as a per-partition bias).
  - softmax is computed without max-subtraction: logits are bounded
    (|f| < ~70 for randn inputs), well within f32/bf16 exp range.

v2 structure:
  - Input DMA / f32->f16 cast / projection matmuls are chunked and
    pipelined so the TensorEngine starts ~8us in instead of ~30us.
  - The softmax denominator d[n] = sum_m exp(fT[m,n]) is accumulated on
    the GPSIMD (Pool) engine as a [128,1024] f32 running sum; only the
    final 128-partition reduction rides the PE (2x512-col ones-matmuls).
    This removes 65k PSUM columns (~27us) from the TensorEngine.
  - 1/d is broadcast across partitions with gpsimd.partition_broadcast.
  - BN statistics: sum(wy) via DVE tensor_tensor_reduce (which also
    materializes wy in SBUF), sum(wy^2) via scalar Square+accum - the two
    run on different engines in parallel.
  - BN scalar math is done for both channel groups at once ([128,2]).
"""

import sys
import numpy as np

if "/opt/trn_rl_repo" not in sys.path:
    sys.path.insert(0, "/opt/trn_rl_repo")

import concourse.bass as bass
import concourse.bacc as bacc
import concourse.mybir as mybir
import concourse.tile as tile
from concourse.bass_utils import run_bass_kernel_spmd

N_CORES = 8
B, C, HW = 4, 256, 64
N = HW * HW          # 4096 spatial positions per batch
IC = 128             # inter channels
NL = N // 2          # 2048 rows per core
NH = NL // 2         # 1024 cols per attention n-half
EPS = 1e-5
CNT = float(B * N)   # BatchNorm count per channel

f32 = mybir.dt.float32
bf16 = mybir.dt.bfloat16
f16 = mybir.dt.float16
ALU = mybir.AluOpType
ACTF = mybir.ActivationFunctionType

import os
USE_GPSIMD_D = os.environ.get("K_GPSIMD_D", "1") == "1"
USE_GPSIMD_BCAST = os.environ.get("K_GPSIMD_BCAST", "1") == "1"
USE_PIPE = os.environ.get("K_PIPE", "1") == "1"          # chunked DMA+cast+proj
# NOTE: tensor_tensor_reduce crashes the device on this runtime (probed:
# both op0=bypass and op0=add variants die) - keep stats on the scalar engine.
USE_TTR = os.environ.get("K_TTR", "0") == "1"
USE_THETA_SC = os.environ.get("K_THETA_SC", "1") == "1"  # theta bias on scalar


def _mm(nc, out, lhsT, rhs, start=True, stop=True):
    return nc.tensor.matmul(out, lhsT, rhs, start=start, stop=stop)


def _build():
    nc = bacc.Bacc("TRN2", target_bir_lowering=False, debug=False,
                   num_devices=N_CORES)

    xl_d = nc.dram_tensor("xl", [C, NL], f32, kind="ExternalInput").ap()
    yl_d = nc.dram_tensor("yl", [C, N], f32, kind="ExternalInput").ap()
    wpk_d = nc.dram_tensor("wpk", [C, 386], f32, kind="ExternalInput").ap()
    wpk2_d = nc.dram_tensor("wpk2", [IC, 257], f32, kind="ExternalInput").ap()
    out_d = nc.dram_tensor("out", [C, NL], f32, kind="ExternalOutput").ap()

    with tile.TileContext(nc) as tc:
        _emit(nc, tc, xl_d, yl_d, wpk_d, wpk2_d, out_d)
    nc.compile()
    return nc


def _emit(nc, tc, xl_d, yl_d, wpk_d, wpk2_d, out_d):
    with (
        tc.tile_pool(name="sb_w", bufs=1) as wp,        # weights + tiny tiles
        tc.tile_pool(name="sb_x", bufs=1) as xp,        # x / y staging
        tc.tile_pool(name="sb_a", bufs=1) as ap_,       # theta/phi/g activations
        tc.tile_pool(name="sb_e", bufs=4) as ep,        # exp tiles
        tc.tile_pool(name="sb_m", bufs=2) as mp,        # misc per-half tiles
        tc.tile_pool(name="sb_bn", bufs=1) as bp,       # bn tiny tiles
        tc.tile_pool(name="ps", bufs=2, space="PSUM") as pp,
        tc.tile_pool(name="dram", bufs=1, space="DRAM") as dr,
    ):
        # ---------------- weights: 3 packed DMAs on the sync queue ----------
        w1 = [wp.tile([128, 386], f32, tag=f"w1_{i}", name=f"w1_{i}")
              for i in range(2)]
        w2 = wp.tile([IC, 257], f32, tag="w2")
        for i in range(2):
            nc.sync.dma_start(w1[i][:], wpk_d[128 * i:128 * (i + 1), :])
        nc.sync.dma_start(w2[:], wpk2_d[:])
        wh1 = [wp.tile([128, 384], f16, tag=f"wh1_{i}", name=f"wh1_{i}")
               for i in range(2)]
        for i in range(2):
            nc.vector.tensor_copy(wh1[i][:], w1[i][:, 0:384])
        wdx_h = [wh1[i][:, 0:128] for i in range(2)]
        wdy_h = [wh1[i][:, 128:256] for i in range(2)]
        wg_h = [wh1[i][:, 256:384] for i in range(2)]
        gamma_t = [w1[i][:, 384:385] for i in range(2)]
        beta_t = [w1[i][:, 385:386] for i in range(2)]
        wwT_b = wp.tile([IC, C], bf16, tag="wwT_b")
        nc.vector.tensor_copy(wwT_b[:], w2[:, 0:256])
        dxb_t = wp.tile([IC, 1], f32, tag="dxb")
        nc.vector.tensor_copy(dxb_t[:], w2[:, 256:257])

        ones_m = wp.tile([128, 1], bf16, tag="ones_m")   # d-matmul stationary
        nc.vector.memset(ones_m[:], 1.0)

        # warmup collective: spins up the ncfw/CC stack during the input-DMA
        # phase so the real stats AllReduces don't pay first-collective
        # latency (cold runs measured +40-90us without this). Runs on the
        # TOPSP/SDMA silicon, fully overlapped with compute; nothing waits
        # on its result.
        warm_sb = wp.tile([1, 1], f32, tag="warm_sb")
        nc.vector.memset(warm_sb[:], 0.0)
        warm_in = dr.tile([1, 1], f32, name="warm_in")
        warm_out = dr.tile([1, 1], f32, name="warm_out")
        nc.sync.dma_start(warm_in[:], warm_sb[:])
        nc.gpsimd.collective_compute(
            "AllReduce", ALU.add,
            replica_groups=[list(range(N_CORES))],
            ins=[warm_in.opt()], outs=[warm_out.opt()])
        ones_mf = wp.tile([128, 1], f32, tag="ones_mf")  # f32r d-reduce stationary
        nc.vector.memset(ones_mf[:], 1.0)
        ones_r = wp.tile([1, 128], f32, tag="ones_r")    # rinv bcast stationary
        nc.vector.memset(ones_r[:], 1.0)

        # ---------------- input staging tiles ----------------
        xl_t = [xp.tile([128, NL], f32, tag=f"xl{c}", bufs=1, name=f"xl{c}")
                for c in range(2)]
        xh_t = [xp.tile([128, NL], f16, tag=f"xh{c}", bufs=1, name=f"xh{c}")
                for c in range(2)]
        # y goes straight to f16 via SWDGE cast-DMA (no f32 staging)
        yh_t = [xp.tile([128, N], f16, tag=f"yh{c}", bufs=1, name=f"yh{c}")
                for c in range(2)]
        theta_h = ap_.tile([IC, NL], f16, tag="theta")
        phi_h = ap_.tile([IC, N], f16, tag="phi")
        g_sb = ap_.tile([128, N], bf16, tag="g")   # 32 chunks [m128, ic128]

        # x DMA (sync queue, t0 first) + casts (vector)
        for t in range(2):
            sl = slice(NH * t, NH * (t + 1))
            for c in range(2):
                nc.sync.dma_start(xl_t[c][:, sl],
                                  xl_d[128 * c:128 * (c + 1), sl])
        for t in range(2):
            sl = slice(NH * t, NH * (t + 1))
            for c in range(2):
                nc.vector.tensor_copy(xh_t[c][:, sl], xl_t[c][:, sl])

        def emit_y_dma(t):
            # f32->f16 cast-DMA on the SWDGE (gpsimd) queue; emitted in-loop
            # so engine order provides prefetch timing.
            sl = slice(1024 * t, 1024 * (t + 1))
            for c in range(2):
                nc.gpsimd.dma_start(yh_t[c][:, sl],
                                    yl_d[128 * c:128 * (c + 1), sl])

        def emit_theta(t):
            ssl = slice(NH * t, NH * (t + 1))
            tp = pp.tile([128, 1024], f32, tag="q", name=f"thp{t}")
            for c in range(2):
                for j in range(2):
                    _mm(nc, tp[:, 512 * j:512 * (j + 1)], wdx_h[c],
                        xh_t[c][:, ssl.start + 512 * j:
                                  ssl.start + 512 * (j + 1)],
                        start=(c == 0), stop=(c == 1))
            if USE_THETA_SC:
                nc.scalar.activation(theta_h[:, ssl], tp[:], ACTF.Identity,
                                     bias=dxb_t[:])
            else:
                nc.vector.tensor_scalar(theta_h[:, ssl], tp[:], dxb_t[:],
                                        None, ALU.add)

        def emit_phig(t):
            # phi + g projections for y chunk t (PE + copies on vector/scalar)
            ssl = slice(1024 * t, 1024 * (t + 1))
            php = pp.tile([128, 1024], f32, tag="q", name=f"php{t}")
            for c in range(2):
                for j in range(2):
                    _mm(nc, php[:, 512 * j:512 * (j + 1)], wdy_h[c],
                        yh_t[c][:, ssl.start + 512 * j:
                                  ssl.start + 512 * (j + 1)],
                        start=(c == 0), stop=(c == 1))
            nc.vector.tensor_copy(phi_h[:, ssl], php[:])
            gp = pp.tile([128, 1024], f32, tag="q", name=f"gp{t}")
            for j in range(8):
                m = ssl.start // 128 + j
                for c in range(2):
                    _mm(nc, gp[:, 128 * j:128 * (j + 1)],
                        yh_t[c][:, 128 * m:128 * (m + 1)], wg_h[c],
                        start=(c == 0), stop=(c == 1))
            nc.scalar.copy(g_sb[:, ssl], gp[:])

        emit_y_dma(0)
        emit_theta(0)
        emit_phig(0)

        # ---------------- attention (cross-half software pipelined) --------
        wy_sb = [mp.tile([128, NL], f16, tag=f"wy{c}", bufs=1, name=f"wy_sb{c}")
                 for c in range(2)]
        # packed stats columns: [c0_sum, c0_sq, c1_sum, c1_sq] per half
        packed = [bp.tile([128, 4], f32, tag=f"packed{h}", name=f"packed{h}")
                  for h in range(2)]
        sums_sc = bp.tile([128, 16], f32, tag="sums_sc")   # scratch columns
        f32r = mybir.dt.float32r
        H = {}      # per-half state
        gstate = {}

        def begin_half(h2):
            s = {}
            s["n0"] = NH * h2
            s["y2"] = [pp.tile([IC, 512], f32, tag=f"y2_{h2}", bufs=2,
                               name=f"y2p{h2}_{j}") for j in range(2)]
            s["dacc_v"] = mp.tile([128, NH], f32r, tag="daccv", bufs=2,
                                  name=f"daccv{h2}")
            s["dacc_g"] = mp.tile([128, NH], f32r, tag="daccg", bufs=2,
                                  name=f"daccg{h2}")
            H[h2] = s
            s["ft"] = emit_f(h2, 0)

        def emit_f(h2, m):
            ft = pp.tile([128, 1024], f32, tag="q", name=f"ft{h2}_{m}")
            for j in range(2):
                _mm(nc, ft[:, 512 * j:512 * (j + 1)],
                    phi_h[:, 128 * m:128 * (m + 1)],
                    theta_h[:, H[h2]["n0"] + 512 * j:
                            H[h2]["n0"] + 512 * (j + 1)])
            return ft

        def emit_iter(h2, m):
            s = H[h2]
            expP = ep.tile([128, 1024], bf16, tag="exp", name=f"ex{h2}_{m}")
            nc.scalar.activation(expP[:], s["ft"][:], ACTF.Exp)
            if h2 == 0:
                if m == 8:
                    emit_theta(1)
                if m in (4, 12, 20):
                    emit_phig(m // 8 + 1)
            if m < 31:
                s["ft"] = emit_f(h2, m + 1)
            for j in range(2):
                _mm(nc, s["y2"][j][:], g_sb[:, 128 * m:128 * (m + 1)],
                    expP[:, 512 * j:512 * (j + 1)],
                    start=(m == 0), stop=(m == 31))
            eng = nc.gpsimd if m % 2 == 0 else nc.vector
            acc = s["dacc_g"] if m % 2 == 0 else s["dacc_v"]
            if m < 2:
                eng.tensor_copy(acc[:], expP[:])
            else:
                eng.tensor_tensor(acc[:], acc[:], expP[:], op=ALU.add)
            if h2 == 0 and m in (2, 10, 18):
                emit_y_dma(m // 8 + 1)

        def emit_dq(h2):
            # d = colsum(dacc_g) + colsum(dacc_v) via accumulated f32r
            # ones-matmuls; then 1/d on DVE.
            s = H[h2]
            dq = pp.tile([128, 1024], f32, tag="q", name=f"dq{h2}")
            for a, acc in enumerate((s["dacc_g"], s["dacc_v"])):
                for j in range(2):
                    _mm(nc, dq[0:1, 512 * j:512 * (j + 1)],
                        ones_mf[:].bitcast(f32r),
                        acc[:, 512 * j:512 * (j + 1)],
                        start=(a == 0), stop=(a == 1))
            rinv = mp.tile([1, NH], f32, tag="rinv", name=f"ri{h2}")
            nc.vector.reciprocal_approx_fast(rinv[:], dq[0:1, :])
            s["rinv"] = rinv

        def emit_norm_wy(h2):
            s = H[h2]
            n0 = s["n0"]
            rb_sb = mp.tile([128, NH], f32, tag="rb", name=f"rb{h2}")
            if USE_GPSIMD_BCAST:
                nc.gpsimd.partition_broadcast(rb_sb[:], s["rinv"][:])
            else:
                rbq = pp.tile([128, 1024], f32, tag="q", name=f"rbq{h2}")
                for j in range(2):
                    _mm(nc, rbq[:, 512 * j:512 * (j + 1)], ones_r[:],
                        s["rinv"][:, 512 * j:512 * (j + 1)])
                nc.vector.tensor_copy(rb_sb[:], rbq[:])
            y2sb = mp.tile([IC, NH], bf16, tag="y2sb", name=f"y2sb{h2}")
            for j in range(2):
                jsl = slice(512 * j, 512 * (j + 1))
                nc.vector.tensor_tensor(y2sb[:, jsl], s["y2"][j][:],
                                        rb_sb[:, jsl], op=ALU.mult)
            # wy: 4 psum tiles reusing this half's y2 banks (freed by the
            # normalize above); stats per 512-chunk: scalar Copy+accum
            # (materializes wy in SBUF) and Square+accum.
            for c in range(2):
                base = 8 * h2 + 4 * c
                for j in range(2):
                    jsl = slice(512 * j, 512 * (j + 1))
                    wyp = pp.tile([128, 512], f32, tag=f"y2_{h2}", bufs=2,
                                  name=f"wyp{h2}_{c}_{j}")
                    _mm(nc, wyp[:], wwT_b[:, 128 * c:128 * (c + 1)],
                        y2sb[:, jsl])
                    nc.scalar.activation(
                        wy_sb[c][:, n0 + 512 * j:n0 + 512 * (j + 1)],
                        wyp[:], ACTF.Copy,
                        accum_out=sums_sc[:, base + j:base + j + 1])
                    sq = ep.tile([128, 512], f16, tag="sqscratch", bufs=2,
                                 name=f"sq{h2}_{c}_{j}")
                    nc.scalar.activation(sq[:], wyp[:], ACTF.Square,
                                         accum_out=sums_sc[:, base + j + 2:
                                                           base + j + 3])
                for t, off in ((0, 0), (1, 2)):
                    nc.vector.tensor_tensor(
                        packed[h2][:, 2 * c + t:2 * c + t + 1],
                        sums_sc[:, base + off:base + off + 1],
                        sums_sc[:, base + off + 1:base + off + 2],
                        op=ALU.add)

        def emit_ar(h2):
            ar_in = dr.tile([128, 4], f32, name=f"ar_in{h2}")
            ar_out = dr.tile([128, 4], f32, name=f"ar_out{h2}")
            # staging DMAs ride the idle sync queue - a gpsimd dma here
            # would block the d-accumulate stream behind AR completion
            nc.sync.dma_start(ar_in[:], packed[h2][:])
            nc.gpsimd.collective_compute(
                "AllReduce", ALU.add,
                replica_groups=[list(range(N_CORES))],
                ins=[ar_in.opt()], outs=[ar_out.opt()])
            gsb = bp.tile([128, 4], f32, tag=f"gsb{h2}", name=f"gsb{h2}")
            nc.sync.dma_start(gsb[:], ar_out[:])
            gstate[h2] = gsb

        K = 8   # h1 iterations emitted before h0's tail chain
        begin_half(0)
        for m in range(32):
            emit_iter(0, m)
        begin_half(1)
        emit_iter(1, 0)
        emit_iter(1, 1)
        emit_dq(0)
        for m in range(2, K):
            emit_iter(1, m)
        emit_norm_wy(0)
        for m in range(K, 12):
            emit_iter(1, m)
        emit_ar(0)
        for m in range(12, 32):
            emit_iter(1, m)
        emit_dq(1)
        emit_norm_wy(1)
        emit_ar(1)

        stats_g = bp.tile([128, 4], f32, tag="stats_g")
        nc.vector.tensor_tensor(stats_g[:], gstate[0][:], gstate[1][:],
                                op=ALU.add)

        # ---------------- BN math + apply + residual ----------------
        for c in range(2):
            mean = bp.tile([128, 1], f32, tag=f"mean{c}")
            nc.vector.tensor_scalar(mean[:], stats_g[:, 2 * c:2 * c + 1],
                                    1.0 / CNT, None, ALU.mult)
            msq = bp.tile([128, 1], f32, tag=f"msq{c}")
            nc.vector.tensor_scalar(msq[:], stats_g[:, 2 * c + 1:2 * c + 2],
                                    1.0 / CNT, None, ALU.mult)
            m2 = bp.tile([128, 1], f32, tag=f"m2{c}")
            nc.vector.tensor_tensor(m2[:], mean[:], mean[:], op=ALU.mult)
            var = bp.tile([128, 1], f32, tag=f"var{c}")
            nc.vector.tensor_tensor(var[:], msq[:], m2[:], op=ALU.subtract)
            varep = bp.tile([128, 1], f32, tag=f"varep{c}")
            nc.vector.tensor_scalar(varep[:], var[:], float(EPS), None, ALU.add)
            sd = bp.tile([128, 1], f32, tag=f"sd{c}")
            nc.scalar.activation(sd[:], varep[:], ACTF.Sqrt)
            rstd = bp.tile([128, 1], f32, tag=f"rstd{c}")
            nc.vector.reciprocal(rstd[:], sd[:])
            scale = bp.tile([128, 1], f32, tag=f"scale{c}")
            nc.vector.tensor_tensor(scale[:], gamma_t[c], rstd[:], op=ALU.mult)
            msc = bp.tile([128, 1], f32, tag=f"msc{c}")
            nc.vector.tensor_tensor(msc[:], mean[:], scale[:], op=ALU.mult)
            shift = bp.tile([128, 1], f32, tag=f"shift{c}")
            nc.vector.tensor_tensor(shift[:], beta_t[c], msc[:], op=ALU.subtract)

            out_t = mp.tile([128, NL], f32, tag=f"out{c}", bufs=1,
                            name=f"out{c}")
            for k in range(2):
                sl = slice(1024 * k, 1024 * (k + 1))
                nc.vector.affine_then_add(out_t[:, sl], wy_sb[c][:, sl],
                                          xl_t[c][:, sl], scale[:], shift[:])
                nc.sync.dma_start(out_d[128 * c:128 * (c + 1), sl],
                                  out_t[:, sl])


_NC_CACHE = None


def _get_nc():
    global _NC_CACHE
    if _NC_CACHE is None:
        _NC_CACHE = _build()
    return _NC_CACHE


def shard_inputs(inputs):
    x = np.ascontiguousarray(inputs["x"], dtype=np.float32).reshape(B, C, N)
    y = np.ascontiguousarray(inputs["y"], dtype=np.float32).reshape(B, C, N)
    dxwT = np.asarray(inputs["dx_w"]).T.astype(np.float32)
    dywT = np.asarray(inputs["dy_w"]).T.astype(np.float32)
    gwT = np.asarray(inputs["g_w"]).T.astype(np.float32)
    wwT = np.asarray(inputs["w_w"]).T.astype(np.float32)
    dxb = np.asarray(inputs["dx_b"], dtype=np.float32).reshape(IC, 1)
    gamma = np.asarray(inputs["bn_gamma"], dtype=np.float32).reshape(C, 1)
    beta = np.asarray(inputs["bn_beta"], dtype=np.float32).reshape(C, 1)
    # pack all small weights into two tensors (3 DMAs instead of 12)
    wpk = np.ascontiguousarray(
        np.concatenate([dxwT, dywT, gwT, gamma, beta], axis=1))   # [256, 386]
    wpk2 = np.ascontiguousarray(
        np.concatenate([wwT, dxb], axis=1))                        # [128, 257]

    in_maps = []
    for core in range(N_CORES):
        b, h = divmod(core, 2)
        in_maps.append({
            "xl": np.ascontiguousarray(x[b][:, h * NL:(h + 1) * NL]),
            "yl": y[b],
            "wpk": wpk, "wpk2": wpk2,
        })
    return in_maps


def run(inputs, **kw):
    """Run on hardware; returns (full_output, BassKernelResults)."""
    nc = _get_nc()
    in_maps = shard_inputs(inputs)
    r = run_bass_kernel_spmd(nc, in_maps, core_ids=list(range(N_CORES)), **kw)
    out = np.empty((B, C, N), np.float32)
    for core in range(N_CORES):
        b, h = divmod(core, 2)
        out[b][:, h * NL:(h + 1) * NL] = r.results[core]["out"]
    return out.reshape(B, C, HW, HW), r


def kernel(**inputs):
    out, _ = run(inputs)
    return out


# revision 46
# speedup vs baseline: 1.0648x; 1.0648x over previous
"""Trainium2 Bass kernel for nn_AttentionBlock (B=4, C=256, H=W=64, IC=128).

Sharding: 8 cores = 4 batches x 2 row-halves of the N=4096 attention dim.
Each core computes its 2048 rows of the attention output, the final 1x1 conv
(wy), and partial BatchNorm statistics; a tiny AllReduce combines the BN
stats; each core then applies BN + residual and writes its output slice.

Algebraic simplifications vs the reference (all exact):
  - g_b and w_b only add a per-channel constant to wy, which BatchNorm's
    mean subtraction cancels -> dropped.
  - dy_b (phi bias) only adds row-constant terms to the attention logits,
    which softmax cancels -> dropped. Only dx_b (theta bias) is applied
    (folded into the theta PSUM->SBUF copy as a per-partition bias).
  - softmax is computed without max-subtraction: logits are bounded
    (|f| < ~70 for randn inputs), well within f32/bf16 exp range.

v2 structure:
  - Input DMA / f32->f16 cast / projection matmuls are chunked and
    pipelined so the TensorEngine starts ~8us in instead of ~30us.
  - The softmax denominator d[n] = sum_m exp(fT[m,n]) is accumulated on
    the GPSIMD (Pool) engine as a [128,1024] f32 running sum; only the
    final 128-partition reduction rides the PE (2x512-col ones-matmuls).
    This removes 65k PSUM columns (~27us) from the TensorEngine.
  - 1/d is broadcast across partitions with gpsimd.partition_broadcast.
  - BN statistics: sum(wy) via DVE tensor_tensor_reduce (which also
    materializes wy in SBUF), sum(wy^2) via scalar Square+accum - the two
    run on different engines in parallel.
  - BN scalar math is done for both channel groups at once ([128,2]).
"""

import sys
import numpy as np

if "/opt/trn_rl_repo" not in sys.path:
    sys.path.insert(0, "/opt/trn_rl_repo")

import concourse.bass as bass
import concourse.bacc as bacc
import concourse.mybir as mybir
import concourse.tile as tile
from concourse.bass_utils import run_bass_kernel_spmd

N_CORES = 8
B, C, HW = 4, 256, 64
N = HW * HW          # 4096 spatial positions per batch
IC = 128             # inter channels
NL = N // 2          # 2048 rows per core
NH = NL // 2         # 1024 cols per attention n-half
EPS = 1e-5
CNT = float(B * N)   # BatchNorm count per channel

f32 = mybir.dt.float32
bf16 = mybir.dt.bfloat16
f16 = mybir.dt.float16
ALU = mybir.AluOpType
ACTF = mybir.ActivationFunctionType

import os
USE_GPSIMD_D = os.environ.get("K_GPSIMD_D", "1") == "1"
USE_GPSIMD_BCAST = os.environ.get("K_GPSIMD_BCAST", "1") == "1"
USE_PIPE = os.environ.get("K_PIPE", "1") == "1"          # chunked DMA+cast+proj
# NOTE: tensor_tensor_reduce crashes the device on this runtime (probed:
# both op0=bypass and op0=add variants die) - keep stats on the scalar engine.
USE_TTR = os.environ.get("K_TTR", "0") == "1"
USE_THETA_SC = os.environ.get("K_THETA_SC", "1") == "1"  # theta bias on scalar


def _mm(nc, out, lhsT, rhs, start=True, stop=True):
    return nc.tensor.matmul(out, lhsT, rhs, start=start, stop=stop)


def _build():
    nc = bacc.Bacc("TRN2", target_bir_lowering=False, debug=False,
                   num_devices=N_CORES)

    xl_d = nc.dram_tensor("xl", [C, NL], f32, kind="ExternalInput").ap()
    yl_d = nc.dram_tensor("yl", [C, N], f32, kind="ExternalInput").ap()
    wpk_d = nc.dram_tensor("wpk", [C, 386], f32, kind="ExternalInput").ap()
    wpk2_d = nc.dram_tensor("wpk2", [IC, 257], f32, kind="ExternalInput").ap()
    out_d = nc.dram_tensor("out", [C, NL], f32, kind="ExternalOutput").ap()

    with tile.TileContext(nc) as tc:
        _emit(nc, tc, xl_d, yl_d, wpk_d, wpk2_d, out_d)
    nc.compile()
    return nc


def _emit(nc, tc, xl_d, yl_d, wpk_d, wpk2_d, out_d):
    with (
        tc.tile_pool(name="sb_w", bufs=1) as wp,        # weights + tiny tiles
        tc.tile_pool(name="sb_x", bufs=1) as xp,        # x / y staging
        tc.tile_pool(name="sb_a", bufs=1) as ap_,       # theta/phi/g activations
        tc.tile_pool(name="sb_e", bufs=4) as ep,        # exp tiles
        tc.tile_pool(name="sb_m", bufs=2) as mp,        # misc per-half tiles
        tc.tile_pool(name="sb_bn", bufs=1) as bp,       # bn tiny tiles
        tc.tile_pool(name="ps", bufs=2, space="PSUM") as pp,
        tc.tile_pool(name="dram", bufs=1, space="DRAM") as dr,
    ):
        # ---------------- weights: 3 packed DMAs on the sync queue ----------
        w1 = [wp.tile([128, 386], f32, tag=f"w1_{i}", name=f"w1_{i}")
              for i in range(2)]
        w2 = wp.tile([IC, 257], f32, tag="w2")
        for i in range(2):
            nc.sync.dma_start(w1[i][:], wpk_d[128 * i:128 * (i + 1), :])
        nc.sync.dma_start(w2[:], wpk2_d[:])
        wh1 = [wp.tile([128, 384], f16, tag=f"wh1_{i}", name=f"wh1_{i}")
               for i in range(2)]
        for i in range(2):
            nc.vector.tensor_copy(wh1[i][:], w1[i][:, 0:384])
        wdx_h = [wh1[i][:, 0:128] for i in range(2)]
        wdy_h = [wh1[i][:, 128:256] for i in range(2)]
        wg_h = [wh1[i][:, 256:384] for i in range(2)]
        gamma_t = [w1[i][:, 384:385] for i in range(2)]
        beta_t = [w1[i][:, 385:386] for i in range(2)]
        wwT_b = wp.tile([IC, C], bf16, tag="wwT_b")
        nc.vector.tensor_copy(wwT_b[:], w2[:, 0:256])
        dxb_t = wp.tile([IC, 1], f32, tag="dxb")
        nc.vector.tensor_copy(dxb_t[:], w2[:, 256:257])

        ones_m = wp.tile([128, 1], bf16, tag="ones_m")   # d-matmul stationary
        nc.vector.memset(ones_m[:], 1.0)
        ones_mf = wp.tile([128, 1], f32, tag="ones_mf")  # f32r d-reduce stationary
        nc.vector.memset(ones_mf[:], 1.0)
        ones_r = wp.tile([1, 128], f32, tag="ones_r")    # rinv bcast stationary
        nc.vector.memset(ones_r[:], 1.0)

        # ---------------- input staging tiles ----------------
        xl_t = [xp.tile([128, NL], f32, tag=f"xl{c}", bufs=1, name=f"xl{c}")
                for c in range(2)]
        xh_t = [xp.tile([128, NL], f16, tag=f"xh{c}", bufs=1, name=f"xh{c}")
                for c in range(2)]
        # y goes straight to f16 via SWDGE cast-DMA (no f32 staging)
        yh_t = [xp.tile([128, N], f16, tag=f"yh{c}", bufs=1, name=f"yh{c}")
                for c in range(2)]
        theta_h = ap_.tile([IC, NL], f16, tag="theta")
        phi_h = ap_.tile([IC, N], f16, tag="phi")
        g_sb = ap_.tile([128, N], bf16, tag="g")   # 32 chunks [m128, ic128]

        # x DMA (sync queue, t0 first) + casts (vector)
        for t in range(2):
            sl = slice(NH * t, NH * (t + 1))
            for c in range(2):
                nc.sync.dma_start(xl_t[c][:, sl],
                                  xl_d[128 * c:128 * (c + 1), sl])
        for t in range(2):
            sl = slice(NH * t, NH * (t + 1))
            for c in range(2):
                nc.vector.tensor_copy(xh_t[c][:, sl], xl_t[c][:, sl])

        def emit_y_dma(t):
            # f32->f16 cast-DMA on the SWDGE (gpsimd) queue; emitted in-loop
            # so engine order provides prefetch timing.
            sl = slice(1024 * t, 1024 * (t + 1))
            for c in range(2):
                nc.gpsimd.dma_start(yh_t[c][:, sl],
                                    yl_d[128 * c:128 * (c + 1), sl])

        def emit_theta(t):
            ssl = slice(NH * t, NH * (t + 1))
            tp = pp.tile([128, 1024], f32, tag="q", name=f"thp{t}")
            for c in range(2):
                for j in range(2):
                    _mm(nc, tp[:, 512 * j:512 * (j + 1)], wdx_h[c],
                        xh_t[c][:, ssl.start + 512 * j:
                                  ssl.start + 512 * (j + 1)],
                        start=(c == 0), stop=(c == 1))
            if USE_THETA_SC:
                nc.scalar.activation(theta_h[:, ssl], tp[:], ACTF.Identity,
                                     bias=dxb_t[:])
            else:
                nc.vector.tensor_scalar(theta_h[:, ssl], tp[:], dxb_t[:],
                                        None, ALU.add)

        def emit_phig(t):
            # phi + g projections for y chunk t (PE + copies on vector/scalar)
            ssl = slice(1024 * t, 1024 * (t + 1))
            php = pp.tile([128, 1024], f32, tag="q", name=f"php{t}")
            for c in range(2):
                for j in range(2):
                    _mm(nc, php[:, 512 * j:512 * (j + 1)], wdy_h[c],
                        yh_t[c][:, ssl.start + 512 * j:
                                  ssl.start + 512 * (j + 1)],
                        start=(c == 0), stop=(c == 1))
            nc.vector.tensor_copy(phi_h[:, ssl], php[:])
            gp = pp.tile([128, 1024], f32, tag="q", name=f"gp{t}")
            for j in range(8):
                m = ssl.start // 128 + j
                for c in range(2):
                    _mm(nc, gp[:, 128 * j:128 * (j + 1)],
                        yh_t[c][:, 128 * m:128 * (m + 1)], wg_h[c],
                        start=(c == 0), stop=(c == 1))
            nc.scalar.copy(g_sb[:, ssl], gp[:])

        emit_y_dma(0)
        emit_theta(0)
        emit_phig(0)

        # ---------------- attention (cross-half software pipelined) --------
        wy_sb = [mp.tile([128, NL], f16, tag=f"wy{c}", bufs=1, name=f"wy_sb{c}")
                 for c in range(2)]
        # packed stats columns: [c0_sum, c0_sq, c1_sum, c1_sq] per half
        packed = [bp.tile([128, 4], f32, tag=f"packed{h}", name=f"packed{h}")
                  for h in range(2)]
        sums_sc = bp.tile([128, 16], f32, tag="sums_sc")   # scratch columns
        f32r = mybir.dt.float32r
        H = {}      # per-half state
        gstate = {}

        def begin_half(h2):
            s = {}
            s["n0"] = NH * h2
            s["y2"] = [pp.tile([IC, 512], f32, tag=f"y2_{h2}", bufs=2,
                               name=f"y2p{h2}_{j}") for j in range(2)]
            s["dacc_v"] = mp.tile([128, NH], f32r, tag="daccv", bufs=2,
                                  name=f"daccv{h2}")
            s["dacc_g"] = mp.tile([128, NH], f32r, tag="daccg", bufs=2,
                                  name=f"daccg{h2}")
            H[h2] = s
            s["ft"] = emit_f(h2, 0)

        def emit_f(h2, m):
            ft = pp.tile([128, 1024], f32, tag="q", name=f"ft{h2}_{m}")
            for j in range(2):
                _mm(nc, ft[:, 512 * j:512 * (j + 1)],
                    phi_h[:, 128 * m:128 * (m + 1)],
                    theta_h[:, H[h2]["n0"] + 512 * j:
                            H[h2]["n0"] + 512 * (j + 1)])
            return ft

        def emit_iter(h2, m):
            s = H[h2]
            expP = ep.tile([128, 1024], bf16, tag="exp", name=f"ex{h2}_{m}")
            nc.scalar.activation(expP[:], s["ft"][:], ACTF.Exp)
            if h2 == 0:
                if m == 8:
                    emit_theta(1)
                if m in (4, 12, 20):
                    emit_phig(m // 8 + 1)
            if m < 31:
                s["ft"] = emit_f(h2, m + 1)
            for j in range(2):
                _mm(nc, s["y2"][j][:], g_sb[:, 128 * m:128 * (m + 1)],
                    expP[:, 512 * j:512 * (j + 1)],
                    start=(m == 0), stop=(m == 31))
            eng = nc.gpsimd if m % 2 == 0 else nc.vector
            acc = s["dacc_g"] if m % 2 == 0 else s["dacc_v"]
            if m < 2:
                eng.tensor_copy(acc[:], expP[:])
            else:
                eng.tensor_tensor(acc[:], acc[:], expP[:], op=ALU.add)
            if h2 == 0 and m in (2, 10, 18):
                emit_y_dma(m // 8 + 1)

        def emit_dq(h2):
            # d = colsum(dacc_g) + colsum(dacc_v) via accumulated f32r
            # ones-matmuls; then 1/d on DVE.
            s = H[h2]
            dq = pp.tile([128, 1024], f32, tag="q", name=f"dq{h2}")
            for a, acc in enumerate((s["dacc_g"], s["dacc_v"])):
                for j in range(2):
                    _mm(nc, dq[0:1, 512 * j:512 * (j + 1)],
                        ones_mf[:].bitcast(f32r),
                        acc[:, 512 * j:512 * (j + 1)],
                        start=(a == 0), stop=(a == 1))
            rinv = mp.tile([1, NH], f32, tag="rinv", name=f"ri{h2}")
            nc.vector.reciprocal_approx_fast(rinv[:], dq[0:1, :])
            s["rinv"] = rinv

        def emit_norm_wy(h2):
            s = H[h2]
            n0 = s["n0"]
            rb_sb = mp.tile([128, NH], f32, tag="rb", name=f"rb{h2}")
            if USE_GPSIMD_BCAST:
                nc.gpsimd.partition_broadcast(rb_sb[:], s["rinv"][:])
            else:
                rbq = pp.tile([128, 1024], f32, tag="q", name=f"rbq{h2}")
                for j in range(2):
                    _mm(nc, rbq[:, 512 * j:512 * (j + 1)], ones_r[:],
                        s["rinv"][:, 512 * j:512 * (j + 1)])
                nc.vector.tensor_copy(rb_sb[:], rbq[:])
            y2sb = mp.tile([IC, NH], bf16, tag="y2sb", name=f"y2sb{h2}")
            for j in range(2):
                jsl = slice(512 * j, 512 * (j + 1))
                nc.vector.tensor_tensor(y2sb[:, jsl], s["y2"][j][:],
                                        rb_sb[:, jsl], op=ALU.mult)
            # wy: 4 psum tiles reusing this half's y2 banks (freed by the
            # normalize above); stats per 512-chunk: scalar Copy+accum
            # (materializes wy in SBUF) and Square+accum.
            for c in range(2):
                base = 8 * h2 + 4 * c
                for j in range(2):
                    jsl = slice(512 * j, 512 * (j + 1))
                    wyp = pp.tile([128, 512], f32, tag=f"y2_{h2}", bufs=2,
                                  name=f"wyp{h2}_{c}_{j}")
                    _mm(nc, wyp[:], wwT_b[:, 128 * c:128 * (c + 1)],
                        y2sb[:, jsl])
                    nc.scalar.activation(
                        wy_sb[c][:, n0 + 512 * j:n0 + 512 * (j + 1)],
                        wyp[:], ACTF.Copy,
                        accum_out=sums_sc[:, base + j:base + j + 1])
                    sq = ep.tile([128, 512], f16, tag="sqscratch", bufs=2,
                                 name=f"sq{h2}_{c}_{j}")
                    nc.scalar.activation(sq[:], wyp[:], ACTF.Square,
                                         accum_out=sums_sc[:, base + j + 2:
                                                           base + j + 3])
                for t, off in ((0, 0), (1, 2)):
                    nc.vector.tensor_tensor(
                        packed[h2][:, 2 * c + t:2 * c + t + 1],
                        sums_sc[:, base + off:base + off + 1],
                        sums_sc[:, base + off + 1:base + off + 2],
                        op=ALU.add)

        def emit_ar(h2):
            ar_in = dr.tile([128, 4], f32, name=f"ar_in{h2}")
            ar_out = dr.tile([128, 4], f32, name=f"ar_out{h2}")
            # staging DMAs ride the idle sync queue - a gpsimd dma here
            # would block the d-accumulate stream behind AR completion
            nc.sync.dma_start(ar_in[:], packed[h2][:])
            nc.gpsimd.collective_compute(
                "AllReduce", ALU.add,
                replica_groups=[list(range(N_CORES))],
                ins=[ar_in.opt()], outs=[ar_out.opt()])
            gsb = bp.tile([128, 4], f32, tag=f"gsb{h2}", name=f"gsb{h2}")
            nc.sync.dma_start(gsb[:], ar_out[:])
            gstate[h2] = gsb

        K = 8   # h1 iterations emitted before h0's tail chain
        begin_half(0)
        for m in range(32):
            emit_iter(0, m)
        begin_half(1)
        emit_iter(1, 0)
        emit_iter(1, 1)
        emit_dq(0)
        for m in range(2, K):
            emit_iter(1, m)
        emit_norm_wy(0)
        for m in range(K, 12):
            emit_iter(1, m)
        emit_ar(0)
        for m in range(12, 32):
            emit_iter(1, m)
        emit_dq(1)
        emit_norm_wy(1)
        emit_ar(1)

        stats_g = bp.tile([128, 4], f32, tag="stats_g")
        nc.vector.tensor_tensor(stats_g[:], gstate[0][:], gstate[1][:],
                                op=ALU.add)

        # ---------------- BN math + apply + residual ----------------
        for c in range(2):
            mean = bp.tile([128, 1], f32, tag=f"mean{c}")
            nc.vector.tensor_scalar(mean[:], stats_g[:, 2 * c:2 * c + 1],
                                    1.0 / CNT, None, ALU.mult)
            msq = bp.tile([128, 1], f32, tag=f"msq{c}")
            nc.vector.tensor_scalar(msq[:], stats_g[:, 2 * c + 1:2 * c + 2],
                                    1.0 / CNT, None, ALU.mult)
            m2 = bp.tile([128, 1], f32, tag=f"m2{c}")
            nc.vector.tensor_tensor(m2[:], mean[:], mean[:], op=ALU.mult)
            var = bp.tile([128, 1], f32, tag=f"var{c}")
            nc.vector.tensor_tensor(var[:], msq[:], m2[:], op=ALU.subtract)
            varep = bp.tile([128, 1], f32, tag=f"varep{c}")
            nc.vector.tensor_scalar(varep[:], var[:], float(EPS), None, ALU.add)
            sd = bp.tile([128, 1], f32, tag=f"sd{c}")
            nc.scalar.activation(sd[:], varep[:], ACTF.Sqrt)
            rstd = bp.tile([128, 1], f32, tag=f"rstd{c}")
            nc.vector.reciprocal(rstd[:], sd[:])
            scale = bp.tile([128, 1], f32, tag=f"scale{c}")
            nc.vector.tensor_tensor(scale[:], gamma_t[c], rstd[:], op=ALU.mult)
            msc = bp.tile([128, 1], f32, tag=f"msc{c}")
            nc.vector.tensor_tensor(msc[:], mean[:], scale[:], op=ALU.mult)
            shift = bp.tile([128, 1], f32, tag=f"shift{c}")
            nc.vector.tensor_tensor(shift[:], beta_t[c], msc[:], op=ALU.subtract)

            out_t = mp.tile([128, NL], f32, tag=f"out{c}", bufs=1,
                            name=f"out{c}")
            for k in range(2):
                sl = slice(1024 * k, 1024 * (k + 1))
                nc.vector.affine_then_add(out_t[:, sl], wy_sb[c][:, sl],
                                          xl_t[c][:, sl], scale[:], shift[:])
                nc.sync.dma_start(out_d[128 * c:128 * (c + 1), sl],
                                  out_t[:, sl])


_NC_CACHE = None


def _get_nc():
    global _NC_CACHE
    if _NC_CACHE is None:
        _NC_CACHE = _build()
    return _NC_CACHE


def shard_inputs(inputs):
    x = np.ascontiguousarray(inputs["x"], dtype=np.float32).reshape(B, C, N)
    y = np.ascontiguousarray(inputs["y"], dtype=np.float32).reshape(B, C, N)
    dxwT = np.asarray(inputs["dx_w"]).T.astype(np.float32)
    dywT = np.asarray(inputs["dy_w"]).T.astype(np.float32)
    gwT = np.asarray(inputs["g_w"]).T.astype(np.float32)
    wwT = np.asarray(inputs["w_w"]).T.astype(np.float32)
    dxb = np.asarray(inputs["dx_b"], dtype=np.float32).reshape(IC, 1)
    gamma = np.asarray(inputs["bn_gamma"], dtype=np.float32).reshape(C, 1)
    beta = np.asarray(inputs["bn_beta"], dtype=np.float32).reshape(C, 1)
    # pack all small weights into two tensors (3 DMAs instead of 12)
    wpk = np.ascontiguousarray(
        np.concatenate([dxwT, dywT, gwT, gamma, beta], axis=1))   # [256, 386]
    wpk2 = np.ascontiguousarray(
        np.concatenate([wwT, dxb], axis=1))                        # [128, 257]

    in_maps = []
    for core in range(N_CORES):
        b, h = divmod(core, 2)
        in_maps.append({
            "xl": np.ascontiguousarray(x[b][:, h * NL:(h + 1) * NL]),
            "yl": y[b],
            "wpk": wpk, "wpk2": wpk2,
        })
    return in_maps


def run(inputs, **kw):
    """Run on hardware; returns (full_output, BassKernelResults)."""
    nc = _get_nc()
    in_maps = shard_inputs(inputs)
    r = run_bass_kernel_spmd(nc, in_maps, core_ids=list(range(N_CORES)), **kw)
    out = np.empty((B, C, N), np.float32)
    for core in range(N_CORES):
        b, h = divmod(core, 2)
        out[b][:, h * NL:(h + 1) * NL] = r.results[core]["out"]
    return out.reshape(B, C, HW, HW), r


def kernel(**inputs):
    out, _ = run(inputs)
    return out
